# revision 6
# baseline (speedup 1.0000x reference)
"""Trainium2 Bass kernel for nn_ConstraintsModule.

Math (reference):
    m = preds[:, atoms]                                   # [B, N]
    body_rev[b,c,j] = pos_body[c,j] + m[b,j]*(neg_body-pos_body)[c,j]
    body_min[b,c]   = 1 - max_j body_rev[b,c,j]
    lb[b,n] = max_c body_min[b,c]*pos_head[c,n]
    ub[b,n] = 1 - max_c body_min[b,c]*neg_head[c,n]
    updated = clamp(m, min(lb,ub), max(lb,ub))
    out = preds with columns `atoms` replaced by updated

Key structure exploited:
  * body masks are ~2% dense (<=40 literals per constraint), so
    max_j body_rev over a 512-wide row is really a max over ~20 gathered
    values of v = [1-m, m, 0]: positive literal j -> v[j] = 1-m[j],
    negative literal j -> v[N+j] = m[j], empty slots -> v[2N] = 0
    (matching the implicit 0 floor of the dense max).
  * each head row is one-hot, so lb/ub are segment-maxes of body_min
    grouped by (head atom, sign).

Sharding: all 128 batch rows live on the SBUF partition axis; the 1024
constraints are sharded across the 8 cores by head atom (core i owns
atoms [64*i, 64*(i+1)) and every constraint whose head lands there).
Gather indices are identical for every batch row, which matches
gpsimd.ap_gather's "same index list per 16-partition group" semantics.

Per-core device program (identical program, per-core data):
  1. DMA in v [128, ~1026], index tensors (int16), m_loc [128, 64].
  2. gpsimd.ap_gather: body values  [128, C_pad*K] from v.
  3. vector.tensor_reduce(max) over K-slot segments -> body_max [128, C_pad].
  4. body_min = 1 - body_max (tensor_scalar), dummy column = 0.
  5. ap_gather body_min by head bins (pos/neg) + segment max -> lbm/ubm.
  6. ub = 1-ubm; updated = max(min(lb,ub), min(max(lb,ub), m_loc)).
  7. DMA out updated [128, 64].
"""

import sys
from contextlib import ExitStack

import numpy as np

if "/opt/trn_rl_repo" not in sys.path:
    sys.path.insert(0, "/opt/trn_rl_repo")

import concourse.bacc as bacc
import concourse.bass as bass
import concourse.tile as tile
from concourse import mybir
from concourse.bass_utils import run_bass_kernel_spmd

B = 128
C = 1024
N = 512
NUM_CLASSES = 1024
NCORES = 8
NLOC = N // NCORES  # atoms per core
V_LEN = 2 * N + 2  # [1-m | m | 0, 0]
PAD_V = 2 * N  # index of a guaranteed-zero slot in v

# Set by test.py to profile; grading path leaves these alone.
_TRACE = False
_LAST_RESULTS = None

_PROGRAM_CACHE: dict = {}


def _roundup(x: int, mult: int) -> int:
    return ((x + mult - 1) // mult) * mult


def _interleave_idx(flat: np.ndarray) -> np.ndarray:
    """Pack a flat int16 index list into the [128, S] SBUF layout that
    ap_gather expects: element e of the list lives at [e % 16, e // 16]
    within each 16-partition group, replicated across the 8 groups."""
    assert flat.ndim == 1 and flat.size % 16 == 0
    grp = flat.reshape(-1, 16).T.astype(np.int16)  # [16, S]
    return np.tile(grp, (8, 1))  # [128, S]


def _build_program(k_pad: int, c_pad: int, lp_pad: int, ln_pad: int):
    key = (k_pad, c_pad, lp_pad, ln_pad)
    if key in _PROGRAM_CACHE:
        return _PROGRAM_CACHE[key]

    dt = mybir.dt
    nb = c_pad * k_pad  # body gather size
    nhp = NLOC * lp_pad  # pos head gather size
    nhn = NLOC * ln_pad
    bm_len = c_pad + 2  # body_min + dummy zero cols
    n_chunks = 4
    # The gather ucode needs its index AP offset 4B-aligned: each chunk's
    # (c_chunk * k_pad) must be a multiple of 32 int16 slots.
    assert c_pad % n_chunks == 0
    c_chunk = c_pad // n_chunks
    assert (c_chunk * k_pad) % 32 == 0

    nc = bacc.Bacc("TRN2", target_bir_lowering=False, debug=False)
    v_d = nc.dram_tensor("v", [B, V_LEN], dt.float32, kind="ExternalInput")
    bidx_d = nc.dram_tensor("bidx", [B, nb // 16], dt.int16, kind="ExternalInput")
    pidx_d = nc.dram_tensor("pidx", [B, nhp // 16], dt.int16, kind="ExternalInput")
    nidx_d = nc.dram_tensor("nidx", [B, nhn // 16], dt.int16, kind="ExternalInput")
    mloc_d = nc.dram_tensor("mloc", [B, NLOC], dt.float32, kind="ExternalInput")
    out_d = nc.dram_tensor("upd", [B, NLOC], dt.float32, kind="ExternalOutput")

    with ExitStack() as ctx:
        tc = ctx.enter_context(tile.TileContext(nc))
        pool = ctx.enter_context(tc.tile_pool(name="main", bufs=1))

        v_sb = pool.tile([B, V_LEN], dt.float32, tag="v")
        nc.sync.dma_start(v_sb[:], v_d.ap())
        bidx_sb = pool.tile([B, nb // 16], dt.int16, tag="bidx")
        nc.sync.dma_start(bidx_sb[:], bidx_d.ap())
        pidx_sb = pool.tile([B, nhp // 16], dt.int16, tag="pidx")
        nc.sync.dma_start(pidx_sb[:], pidx_d.ap())
        nidx_sb = pool.tile([B, nhn // 16], dt.int16, tag="nidx")
        nc.sync.dma_start(nidx_sb[:], nidx_d.ap())
        mloc_sb = pool.tile([B, NLOC], dt.float32, tag="mloc")
        nc.sync.dma_start(mloc_sb[:], mloc_d.ap())

        g_sb = pool.tile([B, nb], dt.float32, tag="g")
        bmax_sb = pool.tile([B, c_pad], dt.float32, tag="bmax")
        bmin_sb = pool.tile([B, bm_len], dt.float32, tag="bmin")

        # Body phase, chunked over constraints so the Pool-engine gather of
        # chunk i+1 overlaps the DVE reduce of chunk i.
        for i in range(n_chunks):
            c0 = i * c_chunk
            e0, e1 = c0 * k_pad, (c0 + c_chunk) * k_pad
            nc.gpsimd.ap_gather(
                g_sb[:, e0:e1],
                v_sb[:],
                bidx_sb[:, e0 // 16 : e1 // 16],
                channels=B,
                num_elems=V_LEN,
                d=1,
                num_idxs=c_chunk * k_pad,
            )
            nc.vector.tensor_reduce(
                bmax_sb[:, c0 : c0 + c_chunk],
                g_sb[:, e0:e1].rearrange("p (c k) -> p c k", k=k_pad),
                axis=mybir.AxisListType.X,
                op=mybir.AluOpType.max,
            )
        # body_min = 1 - body_max; dummy columns stay 0 for empty head bins.
        nc.vector.memset(bmin_sb[:, c_pad:], 0.0)
        nc.vector.tensor_scalar(
            bmin_sb[:, :c_pad],
            bmax_sb[:],
            -1.0,
            1.0,
            op0=mybir.AluOpType.mult,
            op1=mybir.AluOpType.add,
        )

        # Head phase: segment max of body_min grouped by (head atom, sign).
        gp_sb = pool.tile([B, nhp], dt.float32, tag="gp")
        nc.gpsimd.ap_gather(
            gp_sb[:], bmin_sb[:], pidx_sb[:],
            channels=B, num_elems=bm_len, d=1, num_idxs=nhp,
        )
        lb_sb = pool.tile([B, NLOC], dt.float32, tag="lb")
        nc.vector.tensor_reduce(
            lb_sb[:],
            gp_sb[:].rearrange("p (n l) -> p n l", l=lp_pad),
            axis=mybir.AxisListType.X,
            op=mybir.AluOpType.max,
        )
        gn_sb = pool.tile([B, nhn], dt.float32, tag="gn")
        nc.gpsimd.ap_gather(
            gn_sb[:], bmin_sb[:], nidx_sb[:],
            channels=B, num_elems=bm_len, d=1, num_idxs=nhn,
        )
        ubm_sb = pool.tile([B, NLOC], dt.float32, tag="ubm")
        nc.vector.tensor_reduce(
            ubm_sb[:],
            gn_sb[:].rearrange("p (n l) -> p n l", l=ln_pad),
            axis=mybir.AxisListType.X,
            op=mybir.AluOpType.max,
        )

        # updated = max(min(lb, ub), min(max(lb, ub), m))
        ub_sb = pool.tile([B, NLOC], dt.float32, tag="ub")
        nc.vector.tensor_scalar(
            ub_sb[:], ubm_sb[:], -1.0, 1.0,
            op0=mybir.AluOpType.mult, op1=mybir.AluOpType.add,
        )
        lo_sb = pool.tile([B, NLOC], dt.float32, tag="lo")
        nc.vector.tensor_tensor(lo_sb[:], lb_sb[:], ub_sb[:], op=mybir.AluOpType.min)
        hi_sb = pool.tile([B, NLOC], dt.float32, tag="hi")
        nc.vector.tensor_tensor(hi_sb[:], lb_sb[:], ub_sb[:], op=mybir.AluOpType.max)
        upd_sb = pool.tile([B, NLOC], dt.float32, tag="upd")
        nc.vector.tensor_tensor(upd_sb[:], hi_sb[:], mloc_sb[:], op=mybir.AluOpType.min)
        nc.vector.tensor_tensor(upd_sb[:], lo_sb[:], upd_sb[:], op=mybir.AluOpType.max)
        nc.sync.dma_start(out_d.ap(), upd_sb[:])

    nc.compile()
    _PROGRAM_CACHE[key] = nc
    return nc


def kernel(preds, pos_head, neg_head, pos_body, neg_body, atoms):
    global _LAST_RESULTS
    preds = np.ascontiguousarray(np.asarray(preds, dtype=np.float32))
    pos_head = np.asarray(pos_head)
    neg_head = np.asarray(neg_head)
    pos_body = np.asarray(pos_body)
    neg_body = np.asarray(neg_body)
    atoms_np = np.asarray(atoms).astype(np.int64)

    m = preds[:, atoms_np].astype(np.float32)  # [B, N]
    v = np.zeros((B, V_LEN), np.float32)
    v[:, :N] = np.float32(1.0) - m
    v[:, N : 2 * N] = m

    pb = pos_body != 0
    nb_ = neg_body != 0
    # Per-constraint gather targets: pos literal j -> j, neg literal j -> N+j.
    body_counts = pb.sum(1) + nb_.sum(1)
    k_pad = max(_roundup(int(body_counts.max()), 8), 8)
    body_idx = np.full((C, k_pad), PAD_V, np.int16)
    for c in range(C):
        jp = np.nonzero(pb[c])[0]
        jn = np.nonzero(nb_[c])[0]
        body_idx[c, : jp.size] = jp
        body_idx[c, jp.size : jp.size + jn.size] = jn + N

    # Head occurrences (a constraint may have a pos head, a neg head, or both).
    ph_atom = pos_head.argmax(1)
    ph_has = pos_head.max(1) > 0
    nh_atom = neg_head.argmax(1)
    nh_has = neg_head.max(1) > 0

    # Core assignment: every constraint whose pos- or neg-head atom falls in
    # core i's range [NLOC*i, NLOC*(i+1)) joins that core's local list.
    core_lists = []
    for i in range(NCORES):
        lo, hi = NLOC * i, NLOC * (i + 1)
        sel = (ph_has & (ph_atom >= lo) & (ph_atom < hi)) | (
            nh_has & (nh_atom >= lo) & (nh_atom < hi)
        )
        core_lists.append(np.nonzero(sel)[0])
    c_pad = max(_roundup(max(len(x) for x in core_lists), 16), 16)

    # Head bins: local constraint positions grouped by (local atom, sign).
    pos_bins = [[[] for _ in range(NLOC)] for _ in range(NCORES)]
    neg_bins = [[[] for _ in range(NLOC)] for _ in range(NCORES)]
    for i in range(NCORES):
        lo, hi = NLOC * i, NLOC * (i + 1)
        for p_local, cid in enumerate(core_lists[i]):
            if ph_has[cid] and lo <= ph_atom[cid] < hi:
                pos_bins[i][ph_atom[cid] - lo].append(p_local)
            if nh_has[cid] and lo <= nh_atom[cid] < hi:
                neg_bins[i][nh_atom[cid] - lo].append(p_local)
    lp_pad = max(
        _roundup(max(len(b) for bins in pos_bins for b in bins), 4), 4
    )
    ln_pad = max(
        _roundup(max(len(b) for bins in neg_bins for b in bins), 4), 4
    )

    nc = _build_program(k_pad, c_pad, lp_pad, ln_pad)

    in_maps = []
    for i in range(NCORES):
        cids = core_lists[i]
        bflat = np.full((c_pad, k_pad), PAD_V, np.int16)
        bflat[: len(cids)] = body_idx[cids]
        pflat = np.full((NLOC, lp_pad), c_pad, np.int16)  # dummy col -> 0
        nflat = np.full((NLOC, ln_pad), c_pad, np.int16)
        for n_local in range(NLOC):
            for l, p_local in enumerate(pos_bins[i][n_local]):
                pflat[n_local, l] = p_local
            for l, p_local in enumerate(neg_bins[i][n_local]):
                nflat[n_local, l] = p_local
        in_maps.append(
            {
                "v": v,
                "bidx": _interleave_idx(bflat.ravel()),
                "pidx": _interleave_idx(pflat.ravel()),
                "nidx": _interleave_idx(nflat.ravel()),
                "mloc": np.ascontiguousarray(m[:, NLOC * i : NLOC * (i + 1)]),
            }
        )

    res = run_bass_kernel_spmd(
        nc, in_maps, core_ids=list(range(NCORES)), trace=_TRACE
    )
    _LAST_RESULTS = res

    updated = np.concatenate([res.results[i]["upd"] for i in range(NCORES)], axis=1)
    out = preds.copy()
    out[:, atoms_np] = updated
    return out


# revision 9
# speedup vs baseline: 5.1948x; 5.1948x over previous
"""Trainium2 Bass kernel for nn_ConstraintsModule.

Reference math:
    m = preds[:, atoms]                                   # [B, N]
    body_rev[b,c,j] = pos_body[c,j] + m[b,j]*(neg_body-pos_body)[c,j]
    body_min[b,c]   = 1 - max_j body_rev[b,c,j]
    lb[b,n] = max_c body_min[b,c]*pos_head[c,n]
    ub[b,n] = 1 - max_c body_min[b,c]*neg_head[c,n]
    updated = clamp(m, min(lb,ub), max(lb,ub))
    out = preds with columns `atoms` replaced by updated

Structure exploited:
  * body masks are ~2% dense: max_j body_rev is max(1 - min_{j in pos} m,
    max_{j in neg} m) over <=27 literals per constraint (0 if both empty),
    so the 512-wide dense reduction collapses to ~20 packed values.
  * each head row is one-hot: lb/ub are segment maxes of body_min grouped
    by (head atom, sign).

Layout: all 128 batch rows on the SBUF partition axis.  The host packs,
for every constraint slot, its pos-literal m-values (padded with 1.0) and
neg-literal m-values (padded with 0.0) into G[b, slot, 0:K].  Device:
    minP = reduce_min(G[:, :, :Kp]);  maxQ = reduce_max(G[:, :, Kp:])
    body_max = max(1-minP, maxQ);     body_min = 1 - body_max
    lb/ubm   = strided segment maxes of body_min over head bins
    updated  = max(min(lb,ub), min(max(lb,ub), m))        (ub = 1-ubm)
All selection ops (min/max/1-x) round exactly like the reference, so the
result is bit-identical to the fp32 reference.

Sharding: atoms are grouped by (pos-bin-size, neg-bin-size) and dealt
round-robin to the 8 cores, so every core has the *same* group structure
(padded to the max count per group) and the single SPMD program works for
all cores; only the packed data differs.  Each core computes the updated
values for its own atoms; the host scatters them back into preds.
"""

import sys
from contextlib import ExitStack

import numpy as np

if "/opt/trn_rl_repo" not in sys.path:
    sys.path.insert(0, "/opt/trn_rl_repo")

import concourse.bacc as bacc
import concourse.tile as tile
from concourse import mybir
from concourse.bass_utils import run_bass_kernel_spmd

B = 128
C = 1024
N = 512
NCORES = 8

# Set by test.py to profile; the grading path leaves these alone.
_TRACE = False
_LAST_RESULTS = None

_PROGRAM_CACHE: dict = {}


def _roundup(x: int, mult: int) -> int:
    return ((x + mult - 1) // mult) * mult


def _build_program(kp: int, kn: int, s_pad: int, nl_pad: int, groups):
    """groups: tuple of (sp, sn, cnt, col_off, slot_off); identical for all
    cores.  Body slots live at [slot_off + a*(sp+sn)], pos part [0:sp],
    neg part [sp:sp+sn] within each atom's run."""
    key = (kp, kn, s_pad, nl_pad, groups)
    if key in _PROGRAM_CACHE:
        return _PROGRAM_CACHE[key]

    dt = mybir.dt
    k = kp + kn
    n_chunks = 4
    assert s_pad % n_chunks == 0
    s_chunk = s_pad // n_chunks

    nc = bacc.Bacc("TRN2", target_bir_lowering=False, debug=False)
    g_d = nc.dram_tensor("g", [B, s_pad * k], dt.float32, kind="ExternalInput")
    mloc_d = nc.dram_tensor("mloc", [B, nl_pad], dt.float32, kind="ExternalInput")
    out_d = nc.dram_tensor("upd", [B, nl_pad], dt.float32, kind="ExternalOutput")

    with ExitStack() as ctx:
        tc = ctx.enter_context(tile.TileContext(nc))
        pool = ctx.enter_context(tc.tile_pool(name="main", bufs=1))

        mloc_sb = pool.tile([B, nl_pad], dt.float32, tag="mloc")
        nc.sync.dma_start(mloc_sb[:], mloc_d.ap())

        g_sb = pool.tile([B, s_pad * k], dt.float32, tag="g")
        minp_sb = pool.tile([B, s_pad], dt.float32, tag="minp")
        maxq_sb = pool.tile([B, s_pad], dt.float32, tag="maxq")
        dma_engines = [nc.sync, nc.scalar, nc.gpsimd, nc.sync]
        for i in range(n_chunks):
            s0 = i * s_chunk
            e0, e1 = s0 * k, (s0 + s_chunk) * k
            eng = dma_engines[i % len(dma_engines)]
            eng.dma_start(g_sb[:, e0:e1], g_d.ap()[:, e0:e1])
            g3 = g_sb[:, e0:e1].rearrange("p (c k) -> p c k", k=k)
            nc.vector.tensor_reduce(
                minp_sb[:, s0 : s0 + s_chunk],
                g3[:, :, 0:kp],
                axis=mybir.AxisListType.X,
                op=mybir.AluOpType.min,
            )
            nc.vector.tensor_reduce(
                maxq_sb[:, s0 : s0 + s_chunk],
                g3[:, :, kp:k],
                axis=mybir.AxisListType.X,
                op=mybir.AluOpType.max,
            )

        # body_min = 1 - max(1 - minP, maxQ), rounded exactly as the
        # reference (which materializes each 1-m and 1-body_max).
        bmin_sb = pool.tile([B, s_pad], dt.float32, tag="bmin")
        nc.vector.tensor_scalar(
            minp_sb[:], minp_sb[:], -1.0, 1.0,
            op0=mybir.AluOpType.mult, op1=mybir.AluOpType.add,
        )
        nc.vector.tensor_tensor(
            minp_sb[:], minp_sb[:], maxq_sb[:], op=mybir.AluOpType.max
        )
        nc.vector.tensor_scalar(
            bmin_sb[:], minp_sb[:], -1.0, 1.0,
            op0=mybir.AluOpType.mult, op1=mybir.AluOpType.add,
        )

        # Head phase: segment maxes over (atom, sign) bins.
        lb_sb = pool.tile([B, nl_pad], dt.float32, tag="lb")
        ubm_sb = pool.tile([B, nl_pad], dt.float32, tag="ubm")
        nc.vector.memset(lb_sb[:], 0.0)
        nc.vector.memset(ubm_sb[:], 0.0)
        for sp, sn, cnt, col_off, slot_off in groups:
            w = sp + sn
            if w == 0:
                continue  # lb/ubm stay 0 from the memset
            seg = bmin_sb[:, slot_off : slot_off + cnt * w].rearrange(
                "p (a l) -> p a l", l=w
            )
            if sp > 0:
                nc.vector.tensor_reduce(
                    lb_sb[:, col_off : col_off + cnt],
                    seg[:, :, 0:sp],
                    axis=mybir.AxisListType.X,
                    op=mybir.AluOpType.max,
                )
            if sn > 0:
                nc.vector.tensor_reduce(
                    ubm_sb[:, col_off : col_off + cnt],
                    seg[:, :, sp:w],
                    axis=mybir.AxisListType.X,
                    op=mybir.AluOpType.max,
                )

        # updated = max(min(lb, ub), min(max(lb, ub), m)),  ub = 1 - ubm
        ub_sb = pool.tile([B, nl_pad], dt.float32, tag="ub")
        nc.vector.tensor_scalar(
            ub_sb[:], ubm_sb[:], -1.0, 1.0,
            op0=mybir.AluOpType.mult, op1=mybir.AluOpType.add,
        )
        lo_sb = pool.tile([B, nl_pad], dt.float32, tag="lo")
        nc.vector.tensor_tensor(lo_sb[:], lb_sb[:], ub_sb[:], op=mybir.AluOpType.min)
        hi_sb = pool.tile([B, nl_pad], dt.float32, tag="hi")
        nc.vector.tensor_tensor(hi_sb[:], lb_sb[:], ub_sb[:], op=mybir.AluOpType.max)
        upd_sb = pool.tile([B, nl_pad], dt.float32, tag="upd")
        nc.vector.tensor_tensor(upd_sb[:], hi_sb[:], mloc_sb[:], op=mybir.AluOpType.min)
        nc.vector.tensor_tensor(upd_sb[:], lo_sb[:], upd_sb[:], op=mybir.AluOpType.max)
        nc.sync.dma_start(out_d.ap(), upd_sb[:])

    nc.compile()
    _PROGRAM_CACHE[key] = nc
    return nc


def kernel(preds, pos_head, neg_head, pos_body, neg_body, atoms):
    global _LAST_RESULTS
    preds = np.ascontiguousarray(np.asarray(preds, dtype=np.float32))
    pos_head = np.asarray(pos_head)
    neg_head = np.asarray(neg_head)
    pos_body = np.asarray(pos_body)
    neg_body = np.asarray(neg_body)
    atoms_np = np.asarray(atoms).astype(np.int64)

    m = np.ascontiguousarray(preds[:, atoms_np].astype(np.float32))  # [B, N]
    # m_ext columns: [0..N) = m, N = 1.0 (pos pad), N+1 = 0.0 (neg/dummy pad)
    m_ext = np.concatenate(
        [m, np.ones((B, 1), np.float32), np.zeros((B, 1), np.float32)], axis=1
    )
    POS_PAD, NEG_PAD = N, N + 1

    pb = pos_body != 0
    nb_ = neg_body != 0
    kp = max(_roundup(int(pb.sum(1).max()), 4), 4)
    kn = max(_roundup(int(nb_.sum(1).max()), 4), 4)
    k = kp + kn

    # Per-constraint body index rows into m_ext.
    body_idx = np.full((C, k), NEG_PAD, np.int32)
    body_idx[:, :kp] = POS_PAD
    for c in range(C):
        jp = np.nonzero(pb[c])[0]
        jn = np.nonzero(nb_[c])[0]
        body_idx[c, : jp.size] = jp
        body_idx[c, kp : kp + jn.size] = jn

    # Head occurrences: one slot per (constraint, sign) head.
    ph_atom = pos_head.argmax(1)
    ph_has = pos_head.max(1) > 0
    nh_atom = neg_head.argmax(1)
    nh_has = neg_head.max(1) > 0
    pos_bins = [[] for _ in range(N)]
    neg_bins = [[] for _ in range(N)]
    for c in np.nonzero(ph_has)[0]:
        pos_bins[ph_atom[c]].append(c)
    for c in np.nonzero(nh_has)[0]:
        neg_bins[nh_atom[c]].append(c)

    # Group atoms by (sp, sn); deal each group's atoms round-robin to cores.
    from collections import defaultdict

    group_atoms = defaultdict(list)
    for n in range(N):
        group_atoms[(len(pos_bins[n]), len(neg_bins[n]))].append(n)

    groups = []  # (sp, sn, cnt, col_off, slot_off)
    core_atoms = [[] for _ in range(NCORES)]  # (group_idx, pos_in_group, atom)
    col_off = 0
    slot_off = 0
    for (sp, sn), atoms_g in sorted(group_atoms.items()):
        cnt = -(-len(atoms_g) // NCORES)  # ceil
        for j, a in enumerate(atoms_g):
            core_atoms[j % NCORES].append((len(groups), j // NCORES, a))
        groups.append((sp, sn, cnt, col_off, slot_off))
        col_off += cnt
        slot_off += cnt * (sp + sn)
    nl_pad = _roundup(col_off, 4)
    s_pad = _roundup(slot_off, 16)

    nc = _build_program(kp, kn, s_pad, nl_pad, tuple(groups))

    # Per-core packing: slot -> constraint body row (or all-pads for dummy).
    in_maps = []
    out_cols = []  # per core: (cols, atom_ids) to scatter back
    for core in range(NCORES):
        slot_rows = np.full((s_pad, k), NEG_PAD, np.int32)
        mloc_idx = np.full(nl_pad, NEG_PAD, np.int32)  # dummy -> 0.0
        cols = []
        atom_ids = []
        for gi, pos_in_g, a in core_atoms[core]:
            sp, sn, cnt, coff, soff = groups[gi]
            base = soff + pos_in_g * (sp + sn)
            for l, cid in enumerate(pos_bins[a]):
                slot_rows[base + l] = body_idx[cid]
            for l, cid in enumerate(neg_bins[a]):
                slot_rows[base + sp + l] = body_idx[cid]
            mloc_idx[coff + pos_in_g] = a
            cols.append(coff + pos_in_g)
            atom_ids.append(a)
        g_vals = m_ext[:, slot_rows.ravel()]  # [B, s_pad*k]
        mloc = m_ext[:, mloc_idx]  # [B, nl_pad]
        in_maps.append({"g": np.ascontiguousarray(g_vals),
                        "mloc": np.ascontiguousarray(mloc)})
        out_cols.append((np.array(cols), np.array(atom_ids)))

    res = run_bass_kernel_spmd(
        nc, in_maps, core_ids=list(range(NCORES)), trace=_TRACE
    )
    _LAST_RESULTS = res

    out = preds.copy()
    for core in range(NCORES):
        cols, atom_ids = out_cols[core]
        if len(cols):
            out[:, atoms_np[atom_ids]] = res.results[core]["upd"][:, cols]
    return out


# revision 12
# speedup vs baseline: 5.4723x; 1.0534x over previous
"""Trainium2 Bass kernel for nn_ConstraintsModule.

Reference math:
    m = preds[:, atoms]                                   # [B, N]
    body_rev[b,c,j] = pos_body[c,j] + m[b,j]*(neg_body-pos_body)[c,j]
    body_min[b,c]   = 1 - max_j body_rev[b,c,j]
    lb[b,n] = max_c body_min[b,c]*pos_head[c,n]
    ub[b,n] = 1 - max_c body_min[b,c]*neg_head[c,n]
    updated = clamp(m, min(lb,ub), max(lb,ub))
    out = preds with columns `atoms` replaced by updated

Structure exploited:
  * body masks are ~2% dense: max_j body_rev is max(1 - min_{j in pos} m,
    max_{j in neg} m) over <=27 literals per constraint (0 if both empty),
    so the 512-wide dense reduction collapses to ~20 packed values.
  * each head row is one-hot: lb/ub are segment maxes of body_min grouped
    by (head atom, sign).

Layout: all 128 batch rows on the SBUF partition axis.  The host packs,
for every constraint slot, its pos-literal m-values (padded with 1.0) and
neg-literal m-values (padded with 0.0) into G[b, slot, 0:K].  Device:
    minP = reduce_min(G[:, :, :Kp]);  maxQ = reduce_max(G[:, :, Kp:])
    body_max = max(1-minP, maxQ);     body_min = 1 - body_max
    lb/ubm   = strided segment maxes of body_min over head bins
    updated  = max(min(lb,ub), min(max(lb,ub), m))        (ub = 1-ubm)
All selection ops (min/max/1-x) round exactly like the reference, so the
result is bit-identical to the fp32 reference.

Sharding: atoms are grouped by (pos-bin-size, neg-bin-size) and dealt
round-robin to the 8 cores, so every core has the *same* group structure
(padded to the max count per group) and the single SPMD program works for
all cores; only the packed data differs.  Each core computes the updated
values for its own atoms; the host scatters them back into preds.
"""

import sys
from contextlib import ExitStack

import numpy as np

if "/opt/trn_rl_repo" not in sys.path:
    sys.path.insert(0, "/opt/trn_rl_repo")

import concourse.bacc as bacc
import concourse.tile as tile
from concourse import mybir
from concourse.bass_utils import run_bass_kernel_spmd

B = 128
C = 1024
N = 512
NCORES = 8

# Set by test.py to profile; the grading path leaves these alone.
_TRACE = False
_LAST_RESULTS = None

_PROGRAM_CACHE: dict = {}


def _roundup(x: int, mult: int) -> int:
    return ((x + mult - 1) // mult) * mult


def _build_program(kp: int, kn: int, s_pad: int, nl_pad: int, groups):
    """groups: tuple of (sp, sn, cnt, col_off, slot_off); identical for all
    cores.  Body slots live at [slot_off + a*(sp+sn)], pos part [0:sp],
    neg part [sp:sp+sn] within each atom's run."""
    key = (kp, kn, s_pad, nl_pad, groups)
    if key in _PROGRAM_CACHE:
        return _PROGRAM_CACHE[key]

    dt = mybir.dt
    k = kp + kn
    n_chunks = 6
    assert s_pad % n_chunks == 0
    s_chunk = s_pad // n_chunks

    nc = bacc.Bacc("TRN2", target_bir_lowering=False, debug=False)
    # One contiguous DRAM tensor per chunk: column-slicing a single big
    # tensor makes the DMA read DRAM with a large stride (~2x slower).
    g_ds = [
        nc.dram_tensor(f"g{i}", [B, s_chunk * k], dt.float32, kind="ExternalInput")
        for i in range(n_chunks)
    ]
    mloc_d = nc.dram_tensor("mloc", [B, nl_pad], dt.float32, kind="ExternalInput")
    out_d = nc.dram_tensor("upd", [B, nl_pad], dt.float32, kind="ExternalOutput")

    with ExitStack() as ctx:
        tc = ctx.enter_context(tile.TileContext(nc))
        pool = ctx.enter_context(tc.tile_pool(name="main", bufs=1))

        mloc_sb = pool.tile([B, nl_pad], dt.float32, tag="mloc")
        nc.sync.dma_start(mloc_sb[:], mloc_d.ap())

        g_sb = pool.tile([B, s_pad * k], dt.float32, tag="g")
        minp_sb = pool.tile([B, s_pad], dt.float32, tag="minp")
        maxq_sb = pool.tile([B, s_pad], dt.float32, tag="maxq")
        dma_engines = [nc.sync, nc.scalar, nc.gpsimd]
        for i in range(n_chunks):
            s0 = i * s_chunk
            e0, e1 = s0 * k, (s0 + s_chunk) * k
            eng = dma_engines[i % len(dma_engines)]
            eng.dma_start(g_sb[:, e0:e1], g_ds[i].ap())
            g3 = g_sb[:, e0:e1].rearrange("p (c k) -> p c k", k=k)
            nc.vector.tensor_reduce(
                minp_sb[:, s0 : s0 + s_chunk],
                g3[:, :, 0:kp],
                axis=mybir.AxisListType.X,
                op=mybir.AluOpType.min,
            )
            nc.vector.tensor_reduce(
                maxq_sb[:, s0 : s0 + s_chunk],
                g3[:, :, kp:k],
                axis=mybir.AxisListType.X,
                op=mybir.AluOpType.max,
            )

        # body_min = 1 - max(1 - minP, maxQ), rounded exactly as the
        # reference (which materializes each 1-m and 1-body_max).
        bmin_sb = pool.tile([B, s_pad], dt.float32, tag="bmin")
        nc.vector.tensor_scalar(
            minp_sb[:], minp_sb[:], -1.0, 1.0,
            op0=mybir.AluOpType.mult, op1=mybir.AluOpType.add,
        )
        nc.vector.tensor_tensor(
            minp_sb[:], minp_sb[:], maxq_sb[:], op=mybir.AluOpType.max
        )
        nc.vector.tensor_scalar(
            bmin_sb[:], minp_sb[:], -1.0, 1.0,
            op0=mybir.AluOpType.mult, op1=mybir.AluOpType.add,
        )

        # Head phase: segment maxes over (atom, sign) bins.
        lb_sb = pool.tile([B, nl_pad], dt.float32, tag="lb")
        ubm_sb = pool.tile([B, nl_pad], dt.float32, tag="ubm")
        nc.vector.memset(lb_sb[:], 0.0)
        nc.vector.memset(ubm_sb[:], 0.0)
        for sp, sn, cnt, col_off, slot_off in groups:
            w = sp + sn
            if w == 0:
                continue  # lb/ubm stay 0 from the memset
            seg = bmin_sb[:, slot_off : slot_off + cnt * w].rearrange(
                "p (a l) -> p a l", l=w
            )
            if sp > 0:
                nc.vector.tensor_reduce(
                    lb_sb[:, col_off : col_off + cnt],
                    seg[:, :, 0:sp],
                    axis=mybir.AxisListType.X,
                    op=mybir.AluOpType.max,
                )
            if sn > 0:
                nc.vector.tensor_reduce(
                    ubm_sb[:, col_off : col_off + cnt],
                    seg[:, :, sp:w],
                    axis=mybir.AxisListType.X,
                    op=mybir.AluOpType.max,
                )

        # updated = max(min(lb, ub), min(max(lb, ub), m)),  ub = 1 - ubm
        ub_sb = pool.tile([B, nl_pad], dt.float32, tag="ub")
        nc.vector.tensor_scalar(
            ub_sb[:], ubm_sb[:], -1.0, 1.0,
            op0=mybir.AluOpType.mult, op1=mybir.AluOpType.add,
        )
        lo_sb = pool.tile([B, nl_pad], dt.float32, tag="lo")
        nc.vector.tensor_tensor(lo_sb[:], lb_sb[:], ub_sb[:], op=mybir.AluOpType.min)
        hi_sb = pool.tile([B, nl_pad], dt.float32, tag="hi")
        nc.vector.tensor_tensor(hi_sb[:], lb_sb[:], ub_sb[:], op=mybir.AluOpType.max)
        upd_sb = pool.tile([B, nl_pad], dt.float32, tag="upd")
        nc.vector.tensor_tensor(upd_sb[:], hi_sb[:], mloc_sb[:], op=mybir.AluOpType.min)
        nc.vector.tensor_tensor(upd_sb[:], lo_sb[:], upd_sb[:], op=mybir.AluOpType.max)
        nc.sync.dma_start(out_d.ap(), upd_sb[:])

    nc.compile()
    _PROGRAM_CACHE[key] = nc
    return nc


def kernel(preds, pos_head, neg_head, pos_body, neg_body, atoms):
    global _LAST_RESULTS
    preds = np.ascontiguousarray(np.asarray(preds, dtype=np.float32))
    pos_head = np.asarray(pos_head)
    neg_head = np.asarray(neg_head)
    pos_body = np.asarray(pos_body)
    neg_body = np.asarray(neg_body)
    atoms_np = np.asarray(atoms).astype(np.int64)

    m = np.ascontiguousarray(preds[:, atoms_np].astype(np.float32))  # [B, N]
    # m_ext columns: [0..N) = m, N = 1.0 (pos pad), N+1 = 0.0 (neg/dummy pad)
    m_ext = np.concatenate(
        [m, np.ones((B, 1), np.float32), np.zeros((B, 1), np.float32)], axis=1
    )
    POS_PAD, NEG_PAD = N, N + 1

    pb = pos_body != 0
    nb_ = neg_body != 0
    kp = max(_roundup(int(pb.sum(1).max()), 4), 4)
    kn = max(_roundup(int(nb_.sum(1).max()), 4), 4)
    k = kp + kn

    # Per-constraint body index rows into m_ext.
    body_idx = np.full((C, k), NEG_PAD, np.int32)
    body_idx[:, :kp] = POS_PAD
    for c in range(C):
        jp = np.nonzero(pb[c])[0]
        jn = np.nonzero(nb_[c])[0]
        body_idx[c, : jp.size] = jp
        body_idx[c, kp : kp + jn.size] = jn

    # Head occurrences: one slot per (constraint, sign) head.
    ph_atom = pos_head.argmax(1)
    ph_has = pos_head.max(1) > 0
    nh_atom = neg_head.argmax(1)
    nh_has = neg_head.max(1) > 0
    pos_bins = [[] for _ in range(N)]
    neg_bins = [[] for _ in range(N)]
    for c in np.nonzero(ph_has)[0]:
        pos_bins[ph_atom[c]].append(c)
    for c in np.nonzero(nh_has)[0]:
        neg_bins[nh_atom[c]].append(c)

    # Group atoms by (sp, sn); deal each group's atoms round-robin to cores.
    from collections import defaultdict

    group_atoms = defaultdict(list)
    for n in range(N):
        group_atoms[(len(pos_bins[n]), len(neg_bins[n]))].append(n)

    groups = []  # (sp, sn, cnt, col_off, slot_off)
    core_atoms = [[] for _ in range(NCORES)]  # (group_idx, pos_in_group, atom)
    col_off = 0
    slot_off = 0
    for (sp, sn), atoms_g in sorted(group_atoms.items()):
        cnt = -(-len(atoms_g) // NCORES)  # ceil
        for j, a in enumerate(atoms_g):
            core_atoms[j % NCORES].append((len(groups), j // NCORES, a))
        groups.append((sp, sn, cnt, col_off, slot_off))
        col_off += cnt
        slot_off += cnt * (sp + sn)
    nl_pad = _roundup(col_off, 4)
    s_pad = _roundup(slot_off, 48)  # divisible by n_chunks=6 and 16

    nc = _build_program(kp, kn, s_pad, nl_pad, tuple(groups))

    # Per-core packing: slot -> constraint body row (or all-pads for dummy).
    in_maps = []
    out_cols = []  # per core: (cols, atom_ids) to scatter back
    for core in range(NCORES):
        slot_rows = np.full((s_pad, k), NEG_PAD, np.int32)
        mloc_idx = np.full(nl_pad, NEG_PAD, np.int32)  # dummy -> 0.0
        cols = []
        atom_ids = []
        for gi, pos_in_g, a in core_atoms[core]:
            sp, sn, cnt, coff, soff = groups[gi]
            base = soff + pos_in_g * (sp + sn)
            for l, cid in enumerate(pos_bins[a]):
                slot_rows[base + l] = body_idx[cid]
            for l, cid in enumerate(neg_bins[a]):
                slot_rows[base + sp + l] = body_idx[cid]
            mloc_idx[coff + pos_in_g] = a
            cols.append(coff + pos_in_g)
            atom_ids.append(a)
        g_vals = m_ext[:, slot_rows.ravel()]  # [B, s_pad*k]
        mloc = m_ext[:, mloc_idx]  # [B, nl_pad]
        n_chunks = 6
        cw = s_pad * k // n_chunks
        im = {
            f"g{i}": np.ascontiguousarray(g_vals[:, i * cw : (i + 1) * cw])
            for i in range(n_chunks)
        }
        im["mloc"] = np.ascontiguousarray(mloc)
        in_maps.append(im)
        out_cols.append((np.array(cols), np.array(atom_ids)))

    res = run_bass_kernel_spmd(
        nc, in_maps, core_ids=list(range(NCORES)), trace=_TRACE
    )
    _LAST_RESULTS = res

    out = preds.copy()
    for core in range(NCORES):
        cols, atom_ids = out_cols[core]
        if len(cols):
            out[:, atoms_np[atom_ids]] = res.results[core]["upd"][:, cols]
    return out


# revision 13
# speedup vs baseline: 5.6598x; 1.0343x over previous
"""Trainium2 Bass kernel for nn_ConstraintsModule.

Reference math:
    m = preds[:, atoms]                                   # [B, N]
    body_rev[b,c,j] = pos_body[c,j] + m[b,j]*(neg_body-pos_body)[c,j]
    body_min[b,c]   = 1 - max_j body_rev[b,c,j]
    lb[b,n] = max_c body_min[b,c]*pos_head[c,n]
    ub[b,n] = 1 - max_c body_min[b,c]*neg_head[c,n]
    updated = clamp(m, min(lb,ub), max(lb,ub))
    out = preds with columns `atoms` replaced by updated

Structure exploited:
  * body masks are ~2% dense: max_j body_rev collapses to
    max(1 - min_{j in pos} m, max_{j in neg} m) over ~20 literals.
  * head rows are one-hot: lb/ub are segment maxes of body_min grouped by
    (head atom, sign).

Layout: all 128 batch rows on the SBUF partition axis.  The host packs,
per constraint slot, pos-literal m values (padded with 1.0) and
neg-literal m values (padded with 0.0); slots are grouped into a "light"
region (small uniform width) and a "heavy" region (full width) to cut
padding bytes.  Device work is pure DVE: strided tensor_reduce min/max
per region chunk (overlapped with the chunk DMAs), an exact
body_min = 1-max(1-minP, maxQ) rewrite, per-(atom-group) segment maxes,
and the final clamp.  Every op rounds exactly like the reference, so the
result is bit-identical to the fp32 reference.

Sharding: atoms are grouped by (heavy, pos-bin-size, neg-bin-size) and
dealt round-robin to the 8 cores, so all cores share one SPMD program
(groups padded to the cross-core max count); only packed data differs.
"""

import sys
from contextlib import ExitStack

import numpy as np

if "/opt/trn_rl_repo" not in sys.path:
    sys.path.insert(0, "/opt/trn_rl_repo")

import concourse.bacc as bacc
import concourse.tile as tile
from concourse import mybir
from concourse.bass_utils import run_bass_kernel_spmd

B = 128
C = 1024
N = 512
NCORES = 8
N_LIGHT_CHUNKS = 5

# Set by test.py to profile; the grading path leaves these alone.
_TRACE = False
_LAST_RESULTS = None

_PROGRAM_CACHE: dict = {}


def _roundup(x: int, mult: int) -> int:
    return ((x + mult - 1) // mult) * mult


def _build_program(dims, groups):
    """dims = (kpl, knl, kph, knh, sl_pad, sh_pad, nl_pad);
    groups: tuple of (sp, sn, cnt, col_off, slot_off) in the combined slot
    space (light slots first, then heavy)."""
    key = (dims, groups)
    if key in _PROGRAM_CACHE:
        return _PROGRAM_CACHE[key]
    kpl, knl, kph, knh, sl_pad, sh_pad, nl_pad = dims

    dt = mybir.dt
    wl, wh = kpl + knl, kph + knh
    s_tot = sl_pad + sh_pad
    sl_chunk = sl_pad // N_LIGHT_CHUNKS

    nc = bacc.Bacc(
        "TRN2", target_bir_lowering=False, debug=False, enable_partition_id=False
    )
    g_ds = [
        nc.dram_tensor(f"g{i}", [B, sl_chunk * wl], dt.float32, kind="ExternalInput")
        for i in range(N_LIGHT_CHUNKS)
    ]
    gh_d = nc.dram_tensor("gh", [B, max(sh_pad, 1) * wh], dt.float32,
                          kind="ExternalInput")
    mloc_d = nc.dram_tensor("mloc", [B, nl_pad], dt.float32, kind="ExternalInput")
    out_d = nc.dram_tensor("upd", [B, nl_pad], dt.float32, kind="ExternalOutput")

    with ExitStack() as ctx:
        tc = ctx.enter_context(tile.TileContext(nc))
        pool = ctx.enter_context(tc.tile_pool(name="main", bufs=1))

        mloc_sb = pool.tile([B, nl_pad], dt.float32, tag="mloc")
        nc.sync.dma_start(mloc_sb[:], mloc_d.ap())

        gl_sb = pool.tile([B, sl_pad * wl], dt.float32, tag="gl")
        gh_sb = pool.tile([B, max(sh_pad, 1) * wh], dt.float32, tag="gh")
        minp_sb = pool.tile([B, s_tot], dt.float32, tag="minp")
        maxq_sb = pool.tile([B, s_tot], dt.float32, tag="maxq")
        dma_engines = [nc.scalar, nc.gpsimd, nc.sync]
        for i in range(N_LIGHT_CHUNKS):
            s0 = i * sl_chunk
            e0, e1 = s0 * wl, (s0 + sl_chunk) * wl
            dma_engines[i % 3].dma_start(gl_sb[:, e0:e1], g_ds[i].ap())
            g3 = gl_sb[:, e0:e1].rearrange("p (c k) -> p c k", k=wl)
            nc.vector.tensor_reduce(
                minp_sb[:, s0 : s0 + sl_chunk], g3[:, :, 0:kpl],
                axis=mybir.AxisListType.X, op=mybir.AluOpType.min,
            )
            nc.vector.tensor_reduce(
                maxq_sb[:, s0 : s0 + sl_chunk], g3[:, :, kpl:wl],
                axis=mybir.AxisListType.X, op=mybir.AluOpType.max,
            )
        if sh_pad > 0:
            dma_engines[N_LIGHT_CHUNKS % 3].dma_start(gh_sb[:], gh_d.ap())
            g3 = gh_sb[:].rearrange("p (c k) -> p c k", k=wh)
            nc.vector.tensor_reduce(
                minp_sb[:, sl_pad:s_tot], g3[:, :, 0:kph],
                axis=mybir.AxisListType.X, op=mybir.AluOpType.min,
            )
            nc.vector.tensor_reduce(
                maxq_sb[:, sl_pad:s_tot], g3[:, :, kph:wh],
                axis=mybir.AxisListType.X, op=mybir.AluOpType.max,
            )

        # body_min = 1 - max(1 - minP, maxQ), rounded exactly like the
        # reference (which materializes each 1-m and 1-body_max).
        bmin_sb = pool.tile([B, s_tot], dt.float32, tag="bmin")
        nc.vector.tensor_scalar(
            minp_sb[:], minp_sb[:], -1.0, 1.0,
            op0=mybir.AluOpType.mult, op1=mybir.AluOpType.add,
        )
        nc.vector.tensor_tensor(
            minp_sb[:], minp_sb[:], maxq_sb[:], op=mybir.AluOpType.max
        )
        nc.vector.tensor_scalar(
            bmin_sb[:], minp_sb[:], -1.0, 1.0,
            op0=mybir.AluOpType.mult, op1=mybir.AluOpType.add,
        )

        # Head phase: segment maxes over (atom, sign) bins.
        lb_sb = pool.tile([B, nl_pad], dt.float32, tag="lb")
        ubm_sb = pool.tile([B, nl_pad], dt.float32, tag="ubm")
        nc.vector.memset(lb_sb[:], 0.0)
        nc.vector.memset(ubm_sb[:], 0.0)
        for sp, sn, cnt, col_off, slot_off in groups:
            w = sp + sn
            if w == 0:
                continue  # lb/ubm stay 0 from the memset
            seg = bmin_sb[:, slot_off : slot_off + cnt * w].rearrange(
                "p (a l) -> p a l", l=w
            )
            if sp > 0:
                nc.vector.tensor_reduce(
                    lb_sb[:, col_off : col_off + cnt], seg[:, :, 0:sp],
                    axis=mybir.AxisListType.X, op=mybir.AluOpType.max,
                )
            if sn > 0:
                nc.vector.tensor_reduce(
                    ubm_sb[:, col_off : col_off + cnt], seg[:, :, sp:w],
                    axis=mybir.AxisListType.X, op=mybir.AluOpType.max,
                )

        # updated = max(min(lb, ub), min(max(lb, ub), m)),  ub = 1 - ubm
        ub_sb = pool.tile([B, nl_pad], dt.float32, tag="ub")
        nc.vector.tensor_scalar(
            ub_sb[:], ubm_sb[:], -1.0, 1.0,
            op0=mybir.AluOpType.mult, op1=mybir.AluOpType.add,
        )
        lo_sb = pool.tile([B, nl_pad], dt.float32, tag="lo")
        nc.vector.tensor_tensor(lo_sb[:], lb_sb[:], ub_sb[:], op=mybir.AluOpType.min)
        hi_sb = pool.tile([B, nl_pad], dt.float32, tag="hi")
        nc.vector.tensor_tensor(hi_sb[:], lb_sb[:], ub_sb[:], op=mybir.AluOpType.max)
        upd_sb = pool.tile([B, nl_pad], dt.float32, tag="upd")
        nc.vector.tensor_tensor(upd_sb[:], hi_sb[:], mloc_sb[:], op=mybir.AluOpType.min)
        nc.vector.tensor_tensor(upd_sb[:], lo_sb[:], upd_sb[:], op=mybir.AluOpType.max)
        nc.sync.dma_start(out_d.ap(), upd_sb[:])

    nc.compile()
    _PROGRAM_CACHE[key] = nc
    return nc


def kernel(preds, pos_head, neg_head, pos_body, neg_body, atoms):
    global _LAST_RESULTS
    preds = np.ascontiguousarray(np.asarray(preds, dtype=np.float32))
    pos_head = np.asarray(pos_head)
    neg_head = np.asarray(neg_head)
    pos_body = np.asarray(pos_body)
    neg_body = np.asarray(neg_body)
    atoms_np = np.asarray(atoms).astype(np.int64)

    m = np.ascontiguousarray(preds[:, atoms_np].astype(np.float32))  # [B, N]
    # m_ext columns: [0..N) = m, N = 1.0 (pos pad), N+1 = 0.0 (neg/dummy pad)
    m_ext = np.concatenate(
        [m, np.ones((B, 1), np.float32), np.zeros((B, 1), np.float32)], axis=1
    )
    POS_PAD, NEG_PAD = N, N + 1

    pb = pos_body != 0
    nb_ = neg_body != 0
    kp_c = pb.sum(1)
    kn_c = nb_.sum(1)
    kph = max(_roundup(int(kp_c.max()), 4), 4)
    knh = max(_roundup(int(kn_c.max()), 4), 4)

    body_js = [
        (np.nonzero(pb[c])[0], np.nonzero(nb_[c])[0]) for c in range(C)
    ]

    # Head occurrences: one slot per (constraint, sign) head.
    ph_atom = pos_head.argmax(1)
    ph_has = pos_head.max(1) > 0
    nh_atom = neg_head.argmax(1)
    nh_has = neg_head.max(1) > 0
    pos_bins = [[] for _ in range(N)]
    neg_bins = [[] for _ in range(N)]
    for c in np.nonzero(ph_has)[0]:
        pos_bins[ph_atom[c]].append(c)
    for c in np.nonzero(nh_has)[0]:
        neg_bins[nh_atom[c]].append(c)

    # Per-atom max body widths over its bins' constraints.
    atom_kp = np.zeros(N, np.int64)
    atom_kn = np.zeros(N, np.int64)
    for n in range(N):
        cs = pos_bins[n] + neg_bins[n]
        if cs:
            atom_kp[n] = max(kp_c[c] for c in cs)
            atom_kn[n] = max(kn_c[c] for c in cs)

    # Pick the light-tier thresholds minimizing total packed slot bytes.
    best = None
    for kpl in (8, 12, 16, 20, kph):
        for knl in (8, 12, 16, 20, 24, knh):
            light = (atom_kp <= kpl) & (atom_kn <= knl)
            nslots = np.array([len(pos_bins[n]) + len(neg_bins[n]) for n in range(N)])
            cost = (nslots[light].sum() * (kpl + knl)
                    + nslots[~light].sum() * (kph + knh))
            if best is None or cost < best[0]:
                best = (cost, kpl, knl)
    _, kpl, knl = best
    wl, wh = kpl + knl, kph + knh
    atom_heavy = (atom_kp > kpl) | (atom_kn > knl)

    # Group atoms by (heavy, sp, sn); deal round-robin to the 8 cores.
    from collections import defaultdict

    group_atoms = defaultdict(list)
    for n in range(N):
        group_atoms[(bool(atom_heavy[n]), len(pos_bins[n]), len(neg_bins[n]))].append(n)

    # Light groups first: slot index space is [light slots][heavy slots].
    gkeys = sorted(group_atoms)  # False < True
    n_light_slots = sum(
        -(-len(group_atoms[k]) // NCORES) * (k[1] + k[2]) for k in gkeys if not k[0]
    )
    sl_pad = _roundup(max(n_light_slots, N_LIGHT_CHUNKS), N_LIGHT_CHUNKS)

    groups = []  # (sp, sn, cnt, col_off, slot_off) in combined slot space
    core_atoms = [[] for _ in range(NCORES)]  # (group_idx, pos_in_group, atom)
    col_off = 0
    slot_l = 0
    slot_h = sl_pad
    for key in gkeys:
        heavy, sp, sn = key
        atoms_g = group_atoms[key]
        cnt = -(-len(atoms_g) // NCORES)
        for j, a in enumerate(atoms_g):
            core_atoms[j % NCORES].append((len(groups), j // NCORES, a))
        soff = slot_h if heavy else slot_l
        groups.append((sp, sn, cnt, col_off, soff))
        col_off += cnt
        if heavy:
            slot_h += cnt * (sp + sn)
        else:
            slot_l += cnt * (sp + sn)
    assert slot_l <= sl_pad
    sh_pad = slot_h - sl_pad
    nl_pad = _roundup(col_off, 4)

    dims = (kpl, knl, kph, knh, sl_pad, sh_pad, nl_pad)
    nc = _build_program(dims, tuple(groups))

    in_maps = []
    out_cols = []  # per core: (cols, atom_ids) to scatter back
    for core in range(NCORES):
        light_rows = np.full((sl_pad, wl), NEG_PAD, np.int32)
        heavy_rows = np.full((max(sh_pad, 1), wh), NEG_PAD, np.int32)
        mloc_idx = np.full(nl_pad, NEG_PAD, np.int32)  # dummy -> 0.0
        cols = []
        atom_ids = []
        for gi, pos_in_g, a in core_atoms[core]:
            sp, sn, cnt, coff, soff = groups[gi]
            heavy = soff >= sl_pad
            rows, kp_w, base0 = (
                (heavy_rows, kph, soff - sl_pad) if heavy else (light_rows, kpl, soff)
            )
            base = base0 + pos_in_g * (sp + sn)
            for l, cid in enumerate(pos_bins[a] + neg_bins[a]):
                jp, jn = body_js[cid]
                rows[base + l, : jp.size] = jp
                rows[base + l, jp.size : kp_w] = POS_PAD
                rows[base + l, kp_w : kp_w + jn.size] = jn
            mloc_idx[coff + pos_in_g] = a
            cols.append(coff + pos_in_g)
            atom_ids.append(a)
        gl_vals = m_ext[:, light_rows.ravel()]
        gh_vals = m_ext[:, heavy_rows.ravel()]
        cw = sl_pad // N_LIGHT_CHUNKS * wl
        im = {
            f"g{i}": np.ascontiguousarray(gl_vals[:, i * cw : (i + 1) * cw])
            for i in range(N_LIGHT_CHUNKS)
        }
        im["gh"] = np.ascontiguousarray(gh_vals)
        im["mloc"] = np.ascontiguousarray(m_ext[:, mloc_idx])
        in_maps.append(im)
        out_cols.append((np.array(cols), np.array(atom_ids)))

    res = run_bass_kernel_spmd(
        nc, in_maps, core_ids=list(range(NCORES)), trace=_TRACE
    )
    _LAST_RESULTS = res

    out = preds.copy()
    for core in range(NCORES):
        cols, atom_ids = out_cols[core]
        if len(cols):
            out[:, atoms_np[atom_ids]] = res.results[core]["upd"][:, cols]
    return out
